# revision 36
# baseline (speedup 1.0000x reference)
"""Trainium2 Bass kernel for BiGRU(2-layer) + chain-graph GCN(2) + FC.

Strategy (8 NeuronCores, data-parallel over the node dim):
- The GRU layers (seq_len=1, h0=0) are pure per-node gated MLPs.
- The two GCN layers + final FC are linear, so they fuse into a single
  [256 -> 10] projection combined with a 5-point stencil along the node
  dim (weights [1,2,3,2,1]/9 for interior nodes).  The stencil is
  computed as q2 = h2[j] + h2[j+2] on the DVE (4B-aligned -> 2x mode),
  then [1,2,1] taps on q2 plus one direct h2 tap in the projection
  matmul (4 taps instead of 5, and the DVE add is cheap).
- Everything runs feature-major ([feat, node] tiles); x is transposed on
  load via DMA-transpose (bf16).  Matmuls are bf16 with fp32 PSUM accum.
- The output is stored feature-major [10, nodes] and transposed on host
  (saves 128 PE transposes + copies per core).
- Emission is software-pipelined so the scalar (ACT) engine never
  starves: iteration t emits L1(t), then L2(t-1), then stencil(t-2).
  Within a layer, both directions' r/z sigmoids are emitted before
  either direction's tanh, so the DVE STT latency is hidden behind ACT
  work.
- Each core processes 16384 output rows in 16 tiles of 1024 nodes,
  plus a tiny prologue supplying the 4-column h2 halo.  Core c's
  stores cover global rows [c*16384-2, c*16384+16382).
- The 3 first / 3 last rows (graph-boundary degree effects + the 2 rows
  no core computes) are recomputed exactly on host in float64.
"""

import numpy as np
import ml_dtypes

import concourse.bacc as bacc
import concourse.mybir as mybir
import concourse.tile as tile
from concourse import bass_utils

N = 131072
NCORES = 8
PER_CORE = N // NCORES          # 16384
TILE = 1024                     # GRU tile width (nodes)
SUB = 512                       # PSUM bank width (fp32)
NTILES = PER_CORE // TILE       # 16
PROLOG = 128                    # prologue width (h2 halo supplier)
XROWS = PROLOG + PER_CORE       # per-core x shard rows

F32 = mybir.dt.float32
BF16 = mybir.dt.bfloat16
FP8 = mybir.dt.float8e4
AF = mybir.ActivationFunctionType
ALU = mybir.AluOpType
DR = mybir.MatmulPerfMode.DoubleRow
W2SCALE = 32.0  # w2 is stored in fp8 as 32*w; undone by ACT scale=1/32

_cache = {}


def _build_program():
    nc = bacc.Bacc("TRN2", target_bir_lowering=False, debug=False)

    x_d = nc.dram_tensor("x", [XROWS, 128], BF16, kind="ExternalInput")
    w1_d = nc.dram_tensor("w1", [128, 6 * 128], BF16, kind="ExternalInput")
    w2_d = nc.dram_tensor("w2", [128, 12, 128], FP8, kind="ExternalInput")
    wst_d = nc.dram_tensor("wst", [128, 40], BF16, kind="ExternalInput")
    bs_d = nc.dram_tensor("bs", [128, 17], F32, kind="ExternalInput")
    out_d = nc.dram_tensor("out", [10, PER_CORE], F32, kind="ExternalOutput")

    with tile.TileContext(nc) as tc:
        with (
            tc.tile_pool(name="wpool", bufs=1) as wpool,
            tc.tile_pool(name="xpool", bufs=4) as xpool,
            tc.tile_pool(name="gates", bufs=10) as gates,
            tc.tile_pool(name="hpool", bufs=6) as hpool,
            tc.tile_pool(name="h2f", bufs=3) as h2fp,
            tc.tile_pool(name="h2b", bufs=3) as h2bp,
            tc.tile_pool(name="opool", bufs=3) as opool,
            tc.tile_pool(name="qpool", bufs=2) as qpool,
            tc.tile_pool(name="psg", bufs=3, space="PSUM") as psg,
            tc.tile_pool(name="psz", bufs=2, space="PSUM") as psz,
        ):
            # HAM warm-up: dummy matmuls while weight/x DMAs land,
            # so the PE clock-gate is at 8/8 when real work starts.
            junk = wpool.tile([128, 512], BF16)
            nc.gpsimd.memset(junk[:], 0.0)
            jp = psg.tile([128, 512], F32, tag="gi")
            for i in range(16):
                nc.tensor.matmul(jp[:], junk[:, 0:128], junk[:],
                                 start=(i == 0), stop=(i == 15))

            # all parameter loads go on the sync HWDGE queue (fast start,
            # and it keeps the scalar queue free so ACT work starts sooner)
            w1s = wpool.tile([128, 6 * 128], BF16)
            nc.sync.dma_start(out=w1s[:], in_=w1_d.ap())
            bss = wpool.tile([128, 17], F32)
            nc.sync.dma_start(out=bss[:], in_=bs_d.ap())
            w2s = wpool.tile([128, 12, 128], FP8)
            nc.sync.dma_start(out=w2s[:], in_=w2_d.ap())
            wsts = wpool.tile([128, 40], BF16)
            nc.sync.dma_start(out=wsts[:], in_=wst_d.ap())

            # bias column layout in bss: per (layer, dir): r, z, n, hn
            def bcol(l, d, name):
                i = {"r": 0, "z": 1, "n": 2, "hn": 3}[name]
                return bss[:, (l * 2 + d) * 4 + i : (l * 2 + d) * 4 + i + 1]

            def cell_phase1(l, d, rhs, W):
                """W-matmuls + r/z sigmoids + STT (n-gate arg).  Returns
                (zc, s) for phase2.  L1: rhs is a [128, W] bf16 AP.
                L2: rhs is a [128, 2, W] fp8 AP, fed through DoubleRow
                matmuls (K=256 in one instruction, 2 cols/cycle); the
                PSUM carries 32x values (w2 prescale), undone by the
                activation scale."""
                sc = 1.0 if l == 0 else 1.0 / W2SCALE
                gi = [psg.tile([128, W], F32, tag="gi", name=f"gi{g}")
                      for g in range(3)]
                for g in range(3):
                    for n0 in range(0, W, SUB):
                        nw = min(SUB, W - n0)
                        if l == 0:
                            nc.tensor.matmul(
                                gi[g][:, n0 : n0 + nw],
                                w1s[:, (d * 3 + g) * 128 : (d * 3 + g + 1) * 128],
                                rhs[:, n0 : n0 + nw], start=True, stop=True,
                            )
                        else:
                            k = (d * 3 + g) * 2
                            nc.tensor.matmul(
                                gi[g][:, n0 : n0 + nw],
                                w2s[:, k : k + 2, :],
                                rhs[:, :, n0 : n0 + nw], start=True, stop=True,
                                perf_mode=DR,
                            )
                r = gates.tile([128, W], BF16, tag="r")
                nc.scalar.activation(r[:], gi[0][:], AF.Sigmoid,
                                     bias=bcol(l, d, "r"), scale=sc)
                zc = gates.tile([128, W], BF16, tag="zc")
                nc.scalar.activation(zc[:], gi[1][:], AF.Sigmoid,
                                     bias=bcol(l, d, "z"), scale=sc)
                s = gates.tile([128, W], F32, tag="s")
                nc.vector.scalar_tensor_tensor(
                    s[:], r[:], bcol(l, d, "hn"), gi[2][:], ALU.mult, ALU.add
                )
                return zc, s

            def cell_phase2(l, d, zc, s, h_out):
                """tanh + gate combine: h = (1-z) * tanh(n)."""
                sc = 1.0 if l == 0 else 1.0 / W2SCALE
                ng = gates.tile([128, s.shape[1]], BF16, tag="ng")
                nc.scalar.activation(ng[:], s[:], AF.Tanh,
                                     bias=bcol(l, d, "n"), scale=sc)
                nc.vector.tensor_mul(h_out, zc[:], ng[:])

            hist1 = {}  # t -> (h1f, h1b, W)
            hist2 = {}  # t -> (h2f, h2b, fresh_width)

            qhist = {}  # t -> (qf, qb)

            def stencil_pre(t):
                """Halo carry + even-pair add q2[i] = h2[i] + h2[i+2]
                (both operands 4B-aligned in bf16 -> DVE 2x mode)."""
                if t in qhist:
                    return
                h2f, h2b, W = hist2[t]
                ph2f, ph2b, plast = hist2[t - 1]
                # halo carry: last 4 fresh columns of previous step
                nc.vector.tensor_copy(h2f[:, 0:4], ph2f[:, plast : plast + 4])
                nc.vector.tensor_copy(h2b[:, 0:4], ph2b[:, plast : plast + 4])
                qs = []
                for tag, h2 in (("qf", h2f), ("qb", h2b)):
                    q = qpool.tile([128, W + 2], BF16, tag=tag)
                    nc.vector.tensor_add(q[:], h2[:, 0 : W + 2], h2[:, 2 : W + 4])
                    qs.append(q)
                qhist[t] = qs

            def stencil_out(t):
                qf, qb = qhist.pop(t)
                h2f, h2b, _ = hist2[t]
                s10 = opool.tile([10, TILE], F32, tag="s10")
                # z = (1/9) (q2[m] + 2 q2[m+1] + q2[m+2] + h2[m+2]) W
                #   = (1/9) ([1,2,3,2,1] * h2) W
                for u in range(TILE // SUB):
                    z10t = psz.tile([10, SUB], F32, tag="z10", name=f"z10_{u}")
                    z10 = z10t[:]
                    first = True
                    for c, (q, h2) in enumerate(((qf, h2f), (qb, h2b))):
                        for k in range(3):
                            lhsT = wsts[:, 10 * (2 * c + (k == 1)) :
                                        10 * (2 * c + (k == 1)) + 10]
                            nc.tensor.matmul(
                                z10, lhsT,
                                q[:, u * SUB + k : u * SUB + k + SUB],
                                start=first, stop=False,
                            )
                            first = False
                        nc.tensor.matmul(
                            z10, wsts[:, 20 * c : 20 * c + 10],
                            h2[:, u * SUB + 2 : u * SUB + 2 + SUB],
                            start=False, stop=(c == 1),
                        )
                    nc.vector.tensor_scalar_add(
                        s10[:, u * SUB : (u + 1) * SUB], z10, bss[0:10, 16:17])
                nc.gpsimd.dma_start(
                    out=out_d.ap()[:, t * TILE : (t + 1) * TILE], in_=s10[:])

            xtiles = {}

            def load_x(t):
                LW = PROLOG if t < 0 else TILE  # x load width
                r0 = 0 if t < 0 else PROLOG + t * TILE
                xT = xpool.tile([128, LW], BF16, tag="xT")
                nc.sync.dma_start(out=xT[:], in_=x_d.ap()[r0 : r0 + LW, :],
                                  transpose=True)
                xtiles[t] = xT

            def l1(t):
                W = 4 if t < 0 else TILE        # compute width
                xT = xtiles.pop(t)
                LW = xT.shape[1]
                xTc = xT[:, LW - W : LW]  # prologue: last 4 nodes only

                p1f = cell_phase1(0, 0, xTc, W)
                p1b = cell_phase1(0, 1, xTc, W)
                h1t = hpool.tile([128, 2, W], FP8, tag="h1t")
                cell_phase2(0, 0, *p1f, h1t[:, 0, :])
                cell_phase2(0, 1, *p1b, h1t[:, 1, :])
                hist1[t] = (h1t, W)

            def l2(t):
                h1t, W = hist1.pop(t)
                p2f = cell_phase1(1, 0, h1t[:], W)
                p2b = cell_phase1(1, 1, h1t[:], W)
                h2f = h2fp.tile([128, W + 4], BF16, tag="h2f")
                cell_phase2(1, 0, *p2f, h2f[:, 4 : 4 + W])
                h2b = h2bp.tile([128, W + 4], BF16, tag="h2b")
                cell_phase2(1, 1, *p2b, h2b[:, 4 : 4 + W])
                hist2[t] = (h2f, h2b, W)

            load_x(-1)
            load_x(0)
            for t in range(-1, NTILES + 2):
                if t + 1 < NTILES:
                    load_x(t + 1)
                if t < NTILES:
                    l1(t)
                if -1 <= t - 1 < NTILES:
                    l2(t - 1)
                if 0 <= t - 2 < NTILES:
                    stencil_pre(t - 2)
                    stencil_out(t - 2)

    nc.compile()
    return nc


def _prep_inputs(inputs):
    bf = ml_dtypes.bfloat16
    x = np.asarray(inputs["x"], np.float32)

    def pack_l1(wf, wb):
        cols = []
        for w in (wf, wb):
            w = np.asarray(w, np.float32)
            for g in range(3):
                blk = w[g * 128 : (g + 1) * 128, :].T.copy()  # [in, out]
                if g == 1:
                    blk = -blk
                cols.append(blk)
        return np.concatenate(cols, axis=1).astype(bf)  # [128, 768]

    def pack_l2(wf, wb):
        f8 = ml_dtypes.float8_e4m3
        blocks = []
        for w in (wf, wb):
            w = np.asarray(w, np.float32)
            for g in range(3):
                for c in range(2):
                    blk = w[g * 128 : (g + 1) * 128,
                            c * 128 : (c + 1) * 128].T.copy()
                    if g == 1:
                        blk = -blk
                    blocks.append(blk * 32.0)
        return np.stack(blocks, axis=1).astype(f8)  # [128, 12, 128]

    w1 = pack_l1(inputs["w_ih_f1"], inputs["w_ih_b1"])
    w2 = pack_l2(inputs["w_ih_f2"], inputs["w_ih_b2"])

    w_g1 = np.asarray(inputs["w_g1"], np.float32)
    w_g2 = np.asarray(inputs["w_g2"], np.float32)
    w_fc = np.asarray(inputs["w_fc"], np.float32)
    W = w_g1 @ w_g2 @ w_fc  # [256, 10]
    # col blocks: Wf/9, 2Wf/9, Wb/9, 2Wb/9  (q2 taps use [1,2,1], h2 tap 1)
    wst = np.concatenate(
        [W[0:128] / 9.0, W[0:128] * (2.0 / 9.0),
         W[128:256] / 9.0, W[128:256] * (2.0 / 9.0)], axis=1).astype(bf)

    bs = np.zeros((128, 17), np.float32)
    for l, (bi_f, bh_f, bi_b, bh_b) in enumerate((
        (inputs["b_ih_f1"], inputs["b_hh_f1"], inputs["b_ih_b1"], inputs["b_hh_b1"]),
        (inputs["b_ih_f2"], inputs["b_hh_f2"], inputs["b_ih_b2"], inputs["b_hh_b2"]),
    )):
        for d, (bi, bh) in enumerate(((bi_f, bh_f), (bi_b, bh_b))):
            bi = np.asarray(bi, np.float32)
            bh = np.asarray(bh, np.float32)
            base = (l * 2 + d) * 4
            hsc = 1.0 if l == 0 else 32.0  # STT runs in the 32x PSUM domain
            bs[:, base + 0] = bi[0:128] + bh[0:128]
            bs[:, base + 1] = -(bi[128:256] + bh[128:256])
            bs[:, base + 2] = bi[256:384]
            bs[:, base + 3] = bh[256:384] * hsc
    c10 = (np.asarray(inputs["b_g1"], np.float32) @ w_g2 @ w_fc
           + np.asarray(inputs["b_g2"], np.float32) @ w_fc
           + np.asarray(inputs["b_fc"], np.float32))
    bs[0:10, 16] = c10

    xb = x.astype(bf)
    shards = []
    for c in range(NCORES):
        s = c * PER_CORE
        if c == 0:
            xs = np.concatenate(
                [np.zeros((PROLOG, 128), bf), xb[0:PER_CORE]], axis=0)
        else:
            xs = xb[s - PROLOG : s + PER_CORE]
        shards.append(np.ascontiguousarray(xs))

    common = {"w1": w1, "w2": w2, "wst": wst, "bs": bs}
    in_maps = [{"x": shards[c], **common} for c in range(NCORES)]
    return in_maps


def _gru_np(x, w_ih, b_ih, b_hh):
    gi = x @ w_ih.T + b_ih
    ir, iz, inn = gi[:, :128], gi[:, 128:256], gi[:, 256:]
    hr, hz, hn = b_hh[:128], b_hh[128:256], b_hh[256:]
    r = 1.0 / (1.0 + np.exp(-(ir + hr)))
    z = 1.0 / (1.0 + np.exp(-(iz + hz)))
    ng = np.tanh(inn + r * hn)
    return (1.0 - z) * ng


def _fix_boundary(out, inputs, side):
    """Exact (float64) recompute of the 3 boundary rows on one side."""
    M = 16  # margin
    f8 = np.float64
    if side == "left":
        xs = np.asarray(inputs["x"], np.float32)[:M].astype(f8)
    else:
        xs = np.asarray(inputs["x"], np.float32)[-M:].astype(f8)

    def cell(x, tag):
        return _gru_np(x, np.asarray(inputs[f"w_ih_{tag}"], f8),
                       np.asarray(inputs[f"b_ih_{tag}"], f8),
                       np.asarray(inputs[f"b_hh_{tag}"], f8))

    h1 = np.concatenate([cell(xs, "f1"), cell(xs, "b1")], axis=1)
    h2 = np.concatenate([cell(h1, "f2"), cell(h1, "b2")], axis=1)

    c2, c3 = 1.0 / np.sqrt(2.0), 1.0 / np.sqrt(3.0)
    dinv = np.full(M, c3, f8)
    if side == "left":
        dinv[0] = c2
    else:
        dinv[-1] = c2

    def gcn(h, w, b):
        xw = h @ np.asarray(w, f8)
        y = dinv[:, None] * xw
        s = y.copy()
        s[:-1] += y[1:]
        s[1:] += y[:-1]
        return dinv[:, None] * s + np.asarray(b, f8)

    g1 = gcn(h2, inputs["w_g1"], inputs["b_g1"])
    g2 = gcn(g1, inputs["w_g2"], inputs["b_g2"])
    o = g2 @ np.asarray(inputs["w_fc"], f8) + np.asarray(inputs["b_fc"], f8)
    # rows > margin-3 (left) / < 3 from far edge (right) are polluted by the
    # missing neighbour at the margin cut; only the 3 true boundary rows are
    # used, and those only depend on in-margin data.
    if side == "left":
        out[0:3] = o[0:3].astype(np.float32)
    else:
        out[-3:] = o[-3:].astype(np.float32)


def kernel(**inputs):
    if "prog" not in _cache:
        _cache["prog"] = _build_program()
    nc = _cache["prog"]

    in_maps = _prep_inputs(inputs)
    res = bass_utils.run_bass_kernel_spmd(nc, in_maps, core_ids=list(range(NCORES)))

    out = np.empty((N, 10), np.float32)
    for c in range(NCORES):
        shard = np.ascontiguousarray(res.results[c]["out"].T)  # [16384, 10]
        s = c * PER_CORE
        if c == 0:
            out[0 : PER_CORE - 2] = shard[2:]
        else:
            out[s - 2 : s + PER_CORE - 2] = shard
    _fix_boundary(out, inputs, "left")
    _fix_boundary(out, inputs, "right")
    return out


# revision 40
# speedup vs baseline: 1.1878x; 1.1878x over previous
"""Trainium2 Bass kernel for BiGRU(2-layer) + chain-graph GCN(2) + FC.

Strategy (8 NeuronCores, data-parallel over the node dim):
- The GRU layers (seq_len=1, h0=0) are pure per-node gated MLPs.
- The two GCN layers + final FC are linear, so they fuse into a single
  [256 -> 10] projection combined with a 5-point stencil along the node
  dim (weights [1,2,3,2,1]/9 for interior nodes).  The stencil is
  computed as q2 = h2[j] + h2[j+2] on the DVE (4B-aligned -> 2x mode),
  then [1,2,1] taps on q2 plus one direct h2 tap in the projection
  matmul (4 taps instead of 5, and the DVE add is cheap).
- Everything runs feature-major ([feat, node] tiles); x is transposed on
  load via DMA-transpose (bf16).  Matmuls are bf16 with fp32 PSUM accum.
- The output is stored feature-major [10, nodes] and transposed on host
  (saves 128 PE transposes + copies per core).
- Emission is software-pipelined so the scalar (ACT) engine never
  starves: iteration t emits L1(t), then L2(t-1), then stencil(t-2).
  Within a layer, both directions' r/z sigmoids are emitted before
  either direction's tanh, so the DVE STT latency is hidden behind ACT
  work.
- Each core processes 16384 output rows in 16 tiles of 1024 nodes,
  plus a tiny prologue supplying the 4-column h2 halo.  Core c's
  stores cover global rows [c*16384-2, c*16384+16382).
- The 3 first / 3 last rows (graph-boundary degree effects + the 2 rows
  no core computes) are recomputed exactly on host in float64.
"""

import numpy as np
import ml_dtypes

import concourse.bacc as bacc
import concourse.mybir as mybir
import concourse.tile as tile
from concourse import bass_utils

N = 131072
NCORES = 8
PER_CORE = N // NCORES          # 16384
TILE = 1024                     # GRU tile width (nodes)
SUB = 512                       # PSUM bank width (fp32)
NTILES = PER_CORE // TILE       # 16
PROLOG = 128                    # prologue width (h2 halo supplier)
XROWS = PROLOG + PER_CORE       # per-core x shard rows

F32 = mybir.dt.float32
BF16 = mybir.dt.bfloat16
FP8 = mybir.dt.float8e4
AF = mybir.ActivationFunctionType
ALU = mybir.AluOpType
DR = mybir.MatmulPerfMode.DoubleRow
W2SCALE = 32.0  # w2 is stored in fp8 as 32*w; undone by ACT scale=1/32

_cache = {}


def _build_program():
    nc = bacc.Bacc("TRN2", target_bir_lowering=False, debug=False)

    x_d = nc.dram_tensor("x", [XROWS, 128], BF16, kind="ExternalInput")
    w1_d = nc.dram_tensor("w1", [128, 6 * 128], BF16, kind="ExternalInput")
    w2_d = nc.dram_tensor("w2", [128, 12, 128], FP8, kind="ExternalInput")
    wst_d = nc.dram_tensor("wst", [128, 40], BF16, kind="ExternalInput")
    bs_d = nc.dram_tensor("bs", [128, 17], F32, kind="ExternalInput")
    out_d = nc.dram_tensor("out", [10, PER_CORE], F32, kind="ExternalOutput")

    with tile.TileContext(nc) as tc:
        with (
            tc.tile_pool(name="wpool", bufs=1) as wpool,
            tc.tile_pool(name="xpool", bufs=4) as xpool,
            tc.tile_pool(name="gates", bufs=10) as gates,
            tc.tile_pool(name="hpool", bufs=6) as hpool,
            tc.tile_pool(name="h2f", bufs=3) as h2fp,
            tc.tile_pool(name="h2b", bufs=3) as h2bp,
            tc.tile_pool(name="opool", bufs=3) as opool,
            tc.tile_pool(name="qpool", bufs=2) as qpool,
            tc.tile_pool(name="psg", bufs=3, space="PSUM") as psg,
            tc.tile_pool(name="psz", bufs=2, space="PSUM") as psz,
        ):
            # HAM warm-up: dummy matmuls while weight/x DMAs land,
            # so the PE clock-gate is at 8/8 when real work starts.
            junk = wpool.tile([128, 512], BF16)
            nc.gpsimd.memset(junk[:], 0.0)
            jp = psg.tile([128, 512], F32, tag="gi")
            for i in range(6):
                nc.tensor.matmul(jp[:], junk[:, 0:128], junk[:],
                                 start=(i == 0), stop=(i == 5))
            # preload the sigmoid/tanh ACT table sets while input DMAs land,
            # so the ~1.3us table loads are off the critical path
            tact = wpool.tile([128, 4], BF16)
            nc.scalar.activation(tact[:], junk[:, 0:4], AF.Sigmoid)
            nc.scalar.activation(tact[:], junk[:, 0:4], AF.Tanh)

            # all parameter loads go on the sync HWDGE queue (fast start,
            # and it keeps the scalar queue free so ACT work starts sooner)
            w1s = wpool.tile([128, 6 * 128], BF16)
            nc.sync.dma_start(out=w1s[:], in_=w1_d.ap())
            bss = wpool.tile([128, 17], F32)
            nc.sync.dma_start(out=bss[:], in_=bs_d.ap())
            # w2/wst are issued after the first two x loads (see below):
            # they are not needed until l2(-1)/stencil(0)
            w2s = wpool.tile([128, 12, 128], FP8)
            wsts = wpool.tile([128, 40], BF16)

            # bias column layout in bss: per (layer, dir): r, z, n, hn
            def bcol(l, d, name):
                i = {"r": 0, "z": 1, "n": 2, "hn": 3}[name]
                return bss[:, (l * 2 + d) * 4 + i : (l * 2 + d) * 4 + i + 1]

            def cell_phase1(l, d, rhs, W):
                """W-matmuls + r/z sigmoids + STT (n-gate arg).  Returns
                (zc, s) for phase2.  L1: rhs is a [128, W] bf16 AP.
                L2: rhs is a [128, 2, W] fp8 AP, fed through DoubleRow
                matmuls (K=256 in one instruction, 2 cols/cycle); the
                PSUM carries 32x values (w2 prescale), undone by the
                activation scale."""
                sc = 1.0 if l == 0 else 1.0 / W2SCALE
                gi = [psg.tile([128, W], F32, tag="gi", name=f"gi{g}")
                      for g in range(3)]
                for g in range(3):
                    for n0 in range(0, W, SUB):
                        nw = min(SUB, W - n0)
                        if l == 0:
                            nc.tensor.matmul(
                                gi[g][:, n0 : n0 + nw],
                                w1s[:, (d * 3 + g) * 128 : (d * 3 + g + 1) * 128],
                                rhs[:, n0 : n0 + nw], start=True, stop=True,
                            )
                        else:
                            k = (d * 3 + g) * 2
                            nc.tensor.matmul(
                                gi[g][:, n0 : n0 + nw],
                                w2s[:, k : k + 2, :],
                                rhs[:, :, n0 : n0 + nw], start=True, stop=True,
                                perf_mode=DR,
                            )
                r = gates.tile([128, W], BF16, tag="r")
                nc.scalar.activation(r[:], gi[0][:], AF.Sigmoid,
                                     bias=bcol(l, d, "r"), scale=sc)
                zc = gates.tile([128, W], BF16, tag="zc")
                nc.scalar.activation(zc[:], gi[1][:], AF.Sigmoid,
                                     bias=bcol(l, d, "z"), scale=sc)
                s = gates.tile([128, W], F32, tag="s")
                nc.vector.scalar_tensor_tensor(
                    s[:], r[:], bcol(l, d, "hn"), gi[2][:], ALU.mult, ALU.add
                )
                return zc, s

            def cell_phase2(l, d, zc, s, h_out):
                """tanh + gate combine: h = (1-z) * tanh(n)."""
                sc = 1.0 if l == 0 else 1.0 / W2SCALE
                ng = gates.tile([128, s.shape[1]], BF16, tag="ng")
                nc.scalar.activation(ng[:], s[:], AF.Tanh,
                                     bias=bcol(l, d, "n"), scale=sc)
                nc.vector.tensor_mul(h_out, zc[:], ng[:])

            hist1 = {}  # t -> (h1f, h1b, W)
            hist2 = {}  # t -> (h2f, h2b, fresh_width)

            qhist = {}  # t -> (qf, qb)

            def stencil_pre(t):
                """Halo carry + even-pair add q2[i] = h2[i] + h2[i+2]
                (both operands 4B-aligned in bf16 -> DVE 2x mode)."""
                if t in qhist:
                    return
                h2f, h2b, W = hist2[t]
                ph2f, ph2b, plast = hist2[t - 1]
                # halo carry: last 4 fresh columns of previous step
                nc.vector.tensor_copy(h2f[:, 0:4], ph2f[:, plast : plast + 4])
                nc.vector.tensor_copy(h2b[:, 0:4], ph2b[:, plast : plast + 4])
                qs = []
                for tag, h2 in (("qf", h2f), ("qb", h2b)):
                    q = qpool.tile([128, W + 2], BF16, tag=tag)
                    nc.vector.tensor_add(q[:], h2[:, 0 : W + 2], h2[:, 2 : W + 4])
                    qs.append(q)
                qhist[t] = qs

            def stencil_out(t):
                qf, qb = qhist.pop(t)
                h2f, h2b, _ = hist2[t]
                s10 = opool.tile([10, TILE], F32, tag="s10")
                # z = (1/9) (q2[m] + 2 q2[m+1] + q2[m+2] + h2[m+2]) W
                #   = (1/9) ([1,2,3,2,1] * h2) W
                for u in range(TILE // SUB):
                    z10t = psz.tile([10, SUB], F32, tag="z10", name=f"z10_{u}")
                    z10 = z10t[:]
                    first = True
                    for c, (q, h2) in enumerate(((qf, h2f), (qb, h2b))):
                        for k in range(3):
                            lhsT = wsts[:, 10 * (2 * c + (k == 1)) :
                                        10 * (2 * c + (k == 1)) + 10]
                            nc.tensor.matmul(
                                z10, lhsT,
                                q[:, u * SUB + k : u * SUB + k + SUB],
                                start=first, stop=False,
                            )
                            first = False
                        nc.tensor.matmul(
                            z10, wsts[:, 20 * c : 20 * c + 10],
                            h2[:, u * SUB + 2 : u * SUB + 2 + SUB],
                            start=False, stop=(c == 1),
                        )
                    nc.vector.tensor_scalar_add(
                        s10[:, u * SUB : (u + 1) * SUB], z10, bss[0:10, 16:17])
                nc.gpsimd.dma_start(
                    out=out_d.ap()[:, t * TILE : (t + 1) * TILE], in_=s10[:])

            xtiles = {}

            def load_x(t):
                LW = PROLOG if t < 0 else TILE  # x load width
                r0 = 0 if t < 0 else PROLOG + t * TILE
                xT = xpool.tile([128, LW], BF16, tag="xT")
                nc.sync.dma_start(out=xT[:], in_=x_d.ap()[r0 : r0 + LW, :],
                                  transpose=True)
                xtiles[t] = xT

            def l1(t):
                W = 4 if t < 0 else TILE        # compute width
                xT = xtiles.pop(t)
                LW = xT.shape[1]
                xTc = xT[:, LW - W : LW]  # prologue: last 4 nodes only

                p1f = cell_phase1(0, 0, xTc, W)
                p1b = cell_phase1(0, 1, xTc, W)
                h1t = hpool.tile([128, 2, W], FP8, tag="h1t")
                cell_phase2(0, 0, *p1f, h1t[:, 0, :])
                cell_phase2(0, 1, *p1b, h1t[:, 1, :])
                hist1[t] = (h1t, W)

            l2tmp = {}

            def l2_ph1(t):
                h1t, W = hist1.pop(t)
                p2f = cell_phase1(1, 0, h1t[:], W)
                p2b = cell_phase1(1, 1, h1t[:], W)
                l2tmp[t] = (p2f, p2b, W)

            def l2_ph2(t):
                p2f, p2b, W = l2tmp.pop(t)
                h2f = h2fp.tile([128, W + 4], BF16, tag="h2f")
                cell_phase2(1, 0, *p2f, h2f[:, 4 : 4 + W])
                h2b = h2bp.tile([128, W + 4], BF16, tag="h2b")
                cell_phase2(1, 1, *p2b, h2b[:, 4 : 4 + W])
                hist2[t] = (h2f, h2b, W)

            load_x(-1)
            load_x(0)
            nc.sync.dma_start(out=w2s[:], in_=w2_d.ap())
            nc.sync.dma_start(out=wsts[:], in_=wst_d.ap())
            # per iteration: L1(t), L2-matmuls/sigmoids(t-1), stencil(t-2),
            # L2-tanh/combine(t-1).  Sandwiching the stencil between the L2
            # phases keeps the identical PE/ACT instruction order in steady
            # state but lets the final tile's drain chain start earlier.
            for t in range(-1, NTILES + 2):
                if t + 1 < NTILES:
                    load_x(t + 1)
                if t < NTILES:
                    l1(t)
                if -1 <= t - 1 < NTILES:
                    l2_ph1(t - 1)
                if 0 <= t - 2 < NTILES:
                    stencil_pre(t - 2)
                    stencil_out(t - 2)
                if -1 <= t - 1 < NTILES:
                    l2_ph2(t - 1)

    nc.compile()
    return nc


def _prep_inputs(inputs):
    bf = ml_dtypes.bfloat16
    x = np.asarray(inputs["x"], np.float32)

    def pack_l1(wf, wb):
        cols = []
        for w in (wf, wb):
            w = np.asarray(w, np.float32)
            for g in range(3):
                blk = w[g * 128 : (g + 1) * 128, :].T.copy()  # [in, out]
                if g == 1:
                    blk = -blk
                cols.append(blk)
        return np.concatenate(cols, axis=1).astype(bf)  # [128, 768]

    def pack_l2(wf, wb):
        f8 = ml_dtypes.float8_e4m3
        blocks = []
        for w in (wf, wb):
            w = np.asarray(w, np.float32)
            for g in range(3):
                for c in range(2):
                    blk = w[g * 128 : (g + 1) * 128,
                            c * 128 : (c + 1) * 128].T.copy()
                    if g == 1:
                        blk = -blk
                    blocks.append(blk * 32.0)
        return np.stack(blocks, axis=1).astype(f8)  # [128, 12, 128]

    w1 = pack_l1(inputs["w_ih_f1"], inputs["w_ih_b1"])
    w2 = pack_l2(inputs["w_ih_f2"], inputs["w_ih_b2"])

    w_g1 = np.asarray(inputs["w_g1"], np.float32)
    w_g2 = np.asarray(inputs["w_g2"], np.float32)
    w_fc = np.asarray(inputs["w_fc"], np.float32)
    W = w_g1 @ w_g2 @ w_fc  # [256, 10]
    # col blocks: Wf/9, 2Wf/9, Wb/9, 2Wb/9  (q2 taps use [1,2,1], h2 tap 1)
    wst = np.concatenate(
        [W[0:128] / 9.0, W[0:128] * (2.0 / 9.0),
         W[128:256] / 9.0, W[128:256] * (2.0 / 9.0)], axis=1).astype(bf)

    bs = np.zeros((128, 17), np.float32)
    for l, (bi_f, bh_f, bi_b, bh_b) in enumerate((
        (inputs["b_ih_f1"], inputs["b_hh_f1"], inputs["b_ih_b1"], inputs["b_hh_b1"]),
        (inputs["b_ih_f2"], inputs["b_hh_f2"], inputs["b_ih_b2"], inputs["b_hh_b2"]),
    )):
        for d, (bi, bh) in enumerate(((bi_f, bh_f), (bi_b, bh_b))):
            bi = np.asarray(bi, np.float32)
            bh = np.asarray(bh, np.float32)
            base = (l * 2 + d) * 4
            hsc = 1.0 if l == 0 else 32.0  # STT runs in the 32x PSUM domain
            bs[:, base + 0] = bi[0:128] + bh[0:128]
            bs[:, base + 1] = -(bi[128:256] + bh[128:256])
            bs[:, base + 2] = bi[256:384]
            bs[:, base + 3] = bh[256:384] * hsc
    c10 = (np.asarray(inputs["b_g1"], np.float32) @ w_g2 @ w_fc
           + np.asarray(inputs["b_g2"], np.float32) @ w_fc
           + np.asarray(inputs["b_fc"], np.float32))
    bs[0:10, 16] = c10

    xb = x.astype(bf)
    shards = []
    for c in range(NCORES):
        s = c * PER_CORE
        if c == 0:
            xs = np.concatenate(
                [np.zeros((PROLOG, 128), bf), xb[0:PER_CORE]], axis=0)
        else:
            xs = xb[s - PROLOG : s + PER_CORE]
        shards.append(np.ascontiguousarray(xs))

    common = {"w1": w1, "w2": w2, "wst": wst, "bs": bs}
    in_maps = [{"x": shards[c], **common} for c in range(NCORES)]
    return in_maps


def _gru_np(x, w_ih, b_ih, b_hh):
    gi = x @ w_ih.T + b_ih
    ir, iz, inn = gi[:, :128], gi[:, 128:256], gi[:, 256:]
    hr, hz, hn = b_hh[:128], b_hh[128:256], b_hh[256:]
    r = 1.0 / (1.0 + np.exp(-(ir + hr)))
    z = 1.0 / (1.0 + np.exp(-(iz + hz)))
    ng = np.tanh(inn + r * hn)
    return (1.0 - z) * ng


def _fix_boundary(out, inputs, side):
    """Exact (float64) recompute of the 3 boundary rows on one side."""
    M = 16  # margin
    f8 = np.float64
    if side == "left":
        xs = np.asarray(inputs["x"], np.float32)[:M].astype(f8)
    else:
        xs = np.asarray(inputs["x"], np.float32)[-M:].astype(f8)

    def cell(x, tag):
        return _gru_np(x, np.asarray(inputs[f"w_ih_{tag}"], f8),
                       np.asarray(inputs[f"b_ih_{tag}"], f8),
                       np.asarray(inputs[f"b_hh_{tag}"], f8))

    h1 = np.concatenate([cell(xs, "f1"), cell(xs, "b1")], axis=1)
    h2 = np.concatenate([cell(h1, "f2"), cell(h1, "b2")], axis=1)

    c2, c3 = 1.0 / np.sqrt(2.0), 1.0 / np.sqrt(3.0)
    dinv = np.full(M, c3, f8)
    if side == "left":
        dinv[0] = c2
    else:
        dinv[-1] = c2

    def gcn(h, w, b):
        xw = h @ np.asarray(w, f8)
        y = dinv[:, None] * xw
        s = y.copy()
        s[:-1] += y[1:]
        s[1:] += y[:-1]
        return dinv[:, None] * s + np.asarray(b, f8)

    g1 = gcn(h2, inputs["w_g1"], inputs["b_g1"])
    g2 = gcn(g1, inputs["w_g2"], inputs["b_g2"])
    o = g2 @ np.asarray(inputs["w_fc"], f8) + np.asarray(inputs["b_fc"], f8)
    # rows > margin-3 (left) / < 3 from far edge (right) are polluted by the
    # missing neighbour at the margin cut; only the 3 true boundary rows are
    # used, and those only depend on in-margin data.
    if side == "left":
        out[0:3] = o[0:3].astype(np.float32)
    else:
        out[-3:] = o[-3:].astype(np.float32)


def kernel(**inputs):
    if "prog" not in _cache:
        _cache["prog"] = _build_program()
    nc = _cache["prog"]

    in_maps = _prep_inputs(inputs)
    res = bass_utils.run_bass_kernel_spmd(nc, in_maps, core_ids=list(range(NCORES)))

    out = np.empty((N, 10), np.float32)
    for c in range(NCORES):
        shard = np.ascontiguousarray(res.results[c]["out"].T)  # [16384, 10]
        s = c * PER_CORE
        if c == 0:
            out[0 : PER_CORE - 2] = shard[2:]
        else:
            out[s - 2 : s + PER_CORE - 2] = shard
    _fix_boundary(out, inputs, "left")
    _fix_boundary(out, inputs, "right")
    return out


# revision 42
# speedup vs baseline: 1.2133x; 1.0215x over previous
"""Trainium2 Bass kernel for BiGRU(2-layer) + chain-graph GCN(2) + FC.

Strategy (8 NeuronCores, data-parallel over the node dim):
- The GRU layers (seq_len=1, h0=0) are pure per-node gated MLPs.
- The two GCN layers + final FC are linear, so they fuse into a single
  [256 -> 10] projection combined with a 5-point stencil along the node
  dim (weights [1,2,3,2,1]/9 for interior nodes).  The stencil is
  computed as q2 = h2[j] + h2[j+2] on the DVE (4B-aligned -> 2x mode),
  then [1,2,1] taps on q2 plus one direct h2 tap in the projection
  matmul (4 taps instead of 5, and the DVE add is cheap).
- Everything runs feature-major ([feat, node] tiles); x is transposed on
  load via DMA-transpose (bf16).  Matmuls are bf16 with fp32 PSUM accum.
- The output is stored feature-major [10, nodes] and transposed on host
  (saves 128 PE transposes + copies per core).
- Emission is software-pipelined so the scalar (ACT) engine never
  starves: iteration t emits L1(t), then L2(t-1), then stencil(t-2).
  Within a layer, both directions' r/z sigmoids are emitted before
  either direction's tanh, so the DVE STT latency is hidden behind ACT
  work.
- Each core processes 16384 output rows in 16 tiles of 1024 nodes,
  plus a tiny prologue supplying the 4-column h2 halo.  Core c's
  stores cover global rows [c*16384-2, c*16384+16382).
- The 3 first / 3 last rows (graph-boundary degree effects + the 2 rows
  no core computes) are recomputed exactly on host in float64.
"""

import numpy as np
import ml_dtypes

import concourse.bacc as bacc
import concourse.mybir as mybir
import concourse.tile as tile
from concourse import bass_utils

N = 131072
NCORES = 8
PER_CORE = N // NCORES          # 16384
TILE = 1024                     # GRU tile width (nodes)
SUB = 512                       # PSUM bank width (fp32)
NTILES = PER_CORE // TILE       # 16
PROLOG = 128                    # prologue width (h2 halo supplier)
XROWS = PROLOG + PER_CORE       # per-core x shard rows

F32 = mybir.dt.float32
BF16 = mybir.dt.bfloat16
FP8 = mybir.dt.float8e4
AF = mybir.ActivationFunctionType
ALU = mybir.AluOpType
DR = mybir.MatmulPerfMode.DoubleRow
W2SCALE = 32.0  # w2 is stored in fp8 as 32*w; undone by ACT scale=1/32

_cache = {}


def _build_program():
    nc = bacc.Bacc("TRN2", target_bir_lowering=False, debug=False)

    x_d = nc.dram_tensor("x", [XROWS, 128], BF16, kind="ExternalInput")
    w1_d = nc.dram_tensor("w1", [128, 6 * 128], BF16, kind="ExternalInput")
    w2_d = nc.dram_tensor("w2", [128, 12, 128], FP8, kind="ExternalInput")
    wst_d = nc.dram_tensor("wst", [128, 40], BF16, kind="ExternalInput")
    bs_d = nc.dram_tensor("bs", [128, 17], F32, kind="ExternalInput")
    out_d = nc.dram_tensor("out", [10, PER_CORE], F32, kind="ExternalOutput")

    with tile.TileContext(nc) as tc:
        with (
            tc.tile_pool(name="wpool", bufs=1) as wpool,
            tc.tile_pool(name="xpool", bufs=4) as xpool,
            tc.tile_pool(name="gates", bufs=10) as gates,
            tc.tile_pool(name="hpool", bufs=6) as hpool,
            tc.tile_pool(name="h2f", bufs=3) as h2fp,
            tc.tile_pool(name="h2b", bufs=3) as h2bp,
            tc.tile_pool(name="opool", bufs=3) as opool,
            tc.tile_pool(name="qpool", bufs=2) as qpool,
            tc.tile_pool(name="psg", bufs=3, space="PSUM") as psg,
            tc.tile_pool(name="psz", bufs=2, space="PSUM") as psz,
        ):
            # HAM warm-up: dummy matmuls while weight/x DMAs land,
            # so the PE clock-gate is at 8/8 when real work starts.
            junk = wpool.tile([128, 512], BF16)
            nc.gpsimd.memset(junk[:], 0.0)
            jp = psg.tile([128, 512], F32, tag="gi")
            for i in range(4):
                nc.tensor.matmul(jp[:], junk[:, 0:128], junk[:],
                                 start=(i == 0), stop=(i == 3))
            # preload the sigmoid/tanh ACT table sets while input DMAs land,
            # so the ~1.3us table loads are off the critical path
            tact = wpool.tile([128, 4], BF16)
            nc.scalar.activation(tact[:], junk[:, 0:4], AF.Sigmoid)
            nc.scalar.activation(tact[:], junk[:, 0:4], AF.Tanh)

            # all parameter loads go on the sync HWDGE queue (fast start,
            # and it keeps the scalar queue free so ACT work starts sooner)
            w1s = wpool.tile([128, 6 * 128], BF16)
            nc.sync.dma_start(out=w1s[:], in_=w1_d.ap())
            bss = wpool.tile([128, 17], F32)
            nc.sync.dma_start(out=bss[:], in_=bs_d.ap())
            # w2/wst are issued after the first two x loads (see below):
            # they are not needed until l2(-1)/stencil(0)
            w2s = wpool.tile([128, 12, 128], FP8)
            wsts = wpool.tile([128, 40], BF16)

            # bias column layout in bss: per (layer, dir): r, z, n, hn
            def bcol(l, d, name):
                i = {"r": 0, "z": 1, "n": 2, "hn": 3}[name]
                return bss[:, (l * 2 + d) * 4 + i : (l * 2 + d) * 4 + i + 1]

            def cell_phase1(l, d, rhs, W):
                """W-matmuls + r/z sigmoids + STT (n-gate arg).  Returns
                (zc, s) for phase2.  L1: rhs is a [128, W] bf16 AP.
                L2: rhs is a [128, 2, W] fp8 AP, fed through DoubleRow
                matmuls (K=256 in one instruction, 2 cols/cycle); the
                PSUM carries 32x values (w2 prescale), undone by the
                activation scale."""
                sc = 1.0 if l == 0 else 1.0 / W2SCALE
                gi = [psg.tile([128, W], F32, tag="gi", name=f"gi{g}")
                      for g in range(3)]
                for g in range(3):
                    for n0 in range(0, W, SUB):
                        nw = min(SUB, W - n0)
                        if l == 0:
                            nc.tensor.matmul(
                                gi[g][:, n0 : n0 + nw],
                                w1s[:, (d * 3 + g) * 128 : (d * 3 + g + 1) * 128],
                                rhs[:, n0 : n0 + nw], start=True, stop=True,
                            )
                        else:
                            k = (d * 3 + g) * 2
                            nc.tensor.matmul(
                                gi[g][:, n0 : n0 + nw],
                                w2s[:, k : k + 2, :],
                                rhs[:, :, n0 : n0 + nw], start=True, stop=True,
                                perf_mode=DR,
                            )
                r = gates.tile([128, W], BF16, tag="r")
                nc.scalar.activation(r[:], gi[0][:], AF.Sigmoid,
                                     bias=bcol(l, d, "r"), scale=sc)
                zc = gates.tile([128, W], BF16, tag="zc")
                nc.scalar.activation(zc[:], gi[1][:], AF.Sigmoid,
                                     bias=bcol(l, d, "z"), scale=sc)
                s = gates.tile([128, W], F32, tag="s")
                nc.vector.scalar_tensor_tensor(
                    s[:], r[:], bcol(l, d, "hn"), gi[2][:], ALU.mult, ALU.add
                )
                return zc, s

            def cell_phase2(l, d, zc, s, h_out):
                """tanh + gate combine: h = (1-z) * tanh(n)."""
                sc = 1.0 if l == 0 else 1.0 / W2SCALE
                ng = gates.tile([128, s.shape[1]], BF16, tag="ng")
                nc.scalar.activation(ng[:], s[:], AF.Tanh,
                                     bias=bcol(l, d, "n"), scale=sc)
                nc.vector.tensor_mul(h_out, zc[:], ng[:])

            hist1 = {}  # t -> (h1f, h1b, W)
            hist2 = {}  # t -> (h2f, h2b, fresh_width)

            qhist = {}  # t -> (qf, qb)

            def stencil_pre(t):
                """Halo carry + even-pair add q2[i] = h2[i] + h2[i+2]
                (both operands 4B-aligned in bf16 -> DVE 2x mode)."""
                if t in qhist:
                    return
                h2f, h2b, W = hist2[t]
                ph2f, ph2b, plast = hist2[t - 1]
                # halo carry: last 4 fresh columns of previous step
                nc.vector.tensor_copy(h2f[:, 0:4], ph2f[:, plast : plast + 4])
                nc.vector.tensor_copy(h2b[:, 0:4], ph2b[:, plast : plast + 4])
                qs = []
                for tag, h2 in (("qf", h2f), ("qb", h2b)):
                    q = qpool.tile([128, W + 2], BF16, tag=tag)
                    nc.vector.tensor_add(q[:], h2[:, 0 : W + 2], h2[:, 2 : W + 4])
                    qs.append(q)
                qhist[t] = qs

            def stencil_out(t):
                qf, qb = qhist.pop(t)
                h2f, h2b, _ = hist2[t]
                s10 = opool.tile([10, TILE], F32, tag="s10")
                # z = (1/9) (q2[m] + 2 q2[m+1] + q2[m+2] + h2[m+2]) W
                #   = (1/9) ([1,2,3,2,1] * h2) W
                for u in range(TILE // SUB):
                    z10t = psz.tile([10, SUB], F32, tag="z10", name=f"z10_{u}")
                    z10 = z10t[:]
                    first = True
                    for c, (q, h2) in enumerate(((qf, h2f), (qb, h2b))):
                        for k in range(3):
                            lhsT = wsts[:, 10 * (2 * c + (k == 1)) :
                                        10 * (2 * c + (k == 1)) + 10]
                            nc.tensor.matmul(
                                z10, lhsT,
                                q[:, u * SUB + k : u * SUB + k + SUB],
                                start=first, stop=False,
                            )
                            first = False
                        nc.tensor.matmul(
                            z10, wsts[:, 20 * c : 20 * c + 10],
                            h2[:, u * SUB + 2 : u * SUB + 2 + SUB],
                            start=False, stop=(c == 1),
                        )
                    nc.vector.tensor_scalar_add(
                        s10[:, u * SUB : (u + 1) * SUB], z10, bss[0:10, 16:17])
                nc.gpsimd.dma_start(
                    out=out_d.ap()[:, t * TILE : (t + 1) * TILE], in_=s10[:])

            xtiles = {}

            def load_x(t):
                LW = PROLOG if t < 0 else TILE  # x load width
                r0 = 0 if t < 0 else PROLOG + t * TILE
                xT = xpool.tile([128, LW], BF16, tag="xT")
                nc.sync.dma_start(out=xT[:], in_=x_d.ap()[r0 : r0 + LW, :],
                                  transpose=True)
                xtiles[t] = xT

            def l1(t):
                W = 4 if t < 0 else TILE        # compute width
                xT = xtiles.pop(t)
                LW = xT.shape[1]
                xTc = xT[:, LW - W : LW]  # prologue: last 4 nodes only

                p1f = cell_phase1(0, 0, xTc, W)
                p1b = cell_phase1(0, 1, xTc, W)
                h1t = hpool.tile([128, 2, W], FP8, tag="h1t")
                cell_phase2(0, 0, *p1f, h1t[:, 0, :])
                cell_phase2(0, 1, *p1b, h1t[:, 1, :])
                hist1[t] = (h1t, W)

            l2tmp = {}

            def l2_ph1(t):
                h1t, W = hist1.pop(t)
                p2f = cell_phase1(1, 0, h1t[:], W)
                p2b = cell_phase1(1, 1, h1t[:], W)
                l2tmp[t] = (p2f, p2b, W)

            def l2_ph2(t):
                p2f, p2b, W = l2tmp.pop(t)
                h2f = h2fp.tile([128, W + 4], BF16, tag="h2f")
                cell_phase2(1, 0, *p2f, h2f[:, 4 : 4 + W])
                h2b = h2bp.tile([128, W + 4], BF16, tag="h2b")
                cell_phase2(1, 1, *p2b, h2b[:, 4 : 4 + W])
                hist2[t] = (h2f, h2b, W)

            load_x(0)
            load_x(-1)
            nc.sync.dma_start(out=w2s[:], in_=w2_d.ap())
            nc.sync.dma_start(out=wsts[:], in_=wst_d.ap())
            # head: tile 0 goes in front of every queue; the tiny prologue
            # (h2-halo supplier, 4 nodes) follows so it doesn't delay the
            # first full-width activations
            l1(0)
            l1(-1)
            l2_ph1(-1)
            l2_ph2(-1)
            load_x(1)
            # per iteration: L1(t), L2-matmuls/sigmoids(t-1), stencil(t-2),
            # L2-tanh/combine(t-1).  Sandwiching the stencil between the L2
            # phases keeps the identical PE/ACT instruction order in steady
            # state but lets the final tile's drain chain start earlier.
            for t in range(1, NTILES + 2):
                if t + 1 < NTILES:
                    load_x(t + 1)
                if t < NTILES:
                    l1(t)
                if t - 1 < NTILES:
                    l2_ph1(t - 1)
                if 0 <= t - 2 < NTILES:
                    stencil_pre(t - 2)
                    stencil_out(t - 2)
                if t - 1 < NTILES:
                    l2_ph2(t - 1)

    nc.compile()
    return nc


def _prep_inputs(inputs):
    bf = ml_dtypes.bfloat16
    x = np.asarray(inputs["x"], np.float32)

    def pack_l1(wf, wb):
        cols = []
        for w in (wf, wb):
            w = np.asarray(w, np.float32)
            for g in range(3):
                blk = w[g * 128 : (g + 1) * 128, :].T.copy()  # [in, out]
                if g == 1:
                    blk = -blk
                cols.append(blk)
        return np.concatenate(cols, axis=1).astype(bf)  # [128, 768]

    def pack_l2(wf, wb):
        f8 = ml_dtypes.float8_e4m3
        blocks = []
        for w in (wf, wb):
            w = np.asarray(w, np.float32)
            for g in range(3):
                for c in range(2):
                    blk = w[g * 128 : (g + 1) * 128,
                            c * 128 : (c + 1) * 128].T.copy()
                    if g == 1:
                        blk = -blk
                    blocks.append(blk * 32.0)
        return np.stack(blocks, axis=1).astype(f8)  # [128, 12, 128]

    w1 = pack_l1(inputs["w_ih_f1"], inputs["w_ih_b1"])
    w2 = pack_l2(inputs["w_ih_f2"], inputs["w_ih_b2"])

    w_g1 = np.asarray(inputs["w_g1"], np.float32)
    w_g2 = np.asarray(inputs["w_g2"], np.float32)
    w_fc = np.asarray(inputs["w_fc"], np.float32)
    W = w_g1 @ w_g2 @ w_fc  # [256, 10]
    # col blocks: Wf/9, 2Wf/9, Wb/9, 2Wb/9  (q2 taps use [1,2,1], h2 tap 1)
    wst = np.concatenate(
        [W[0:128] / 9.0, W[0:128] * (2.0 / 9.0),
         W[128:256] / 9.0, W[128:256] * (2.0 / 9.0)], axis=1).astype(bf)

    bs = np.zeros((128, 17), np.float32)
    for l, (bi_f, bh_f, bi_b, bh_b) in enumerate((
        (inputs["b_ih_f1"], inputs["b_hh_f1"], inputs["b_ih_b1"], inputs["b_hh_b1"]),
        (inputs["b_ih_f2"], inputs["b_hh_f2"], inputs["b_ih_b2"], inputs["b_hh_b2"]),
    )):
        for d, (bi, bh) in enumerate(((bi_f, bh_f), (bi_b, bh_b))):
            bi = np.asarray(bi, np.float32)
            bh = np.asarray(bh, np.float32)
            base = (l * 2 + d) * 4
            hsc = 1.0 if l == 0 else 32.0  # STT runs in the 32x PSUM domain
            bs[:, base + 0] = bi[0:128] + bh[0:128]
            bs[:, base + 1] = -(bi[128:256] + bh[128:256])
            bs[:, base + 2] = bi[256:384]
            bs[:, base + 3] = bh[256:384] * hsc
    c10 = (np.asarray(inputs["b_g1"], np.float32) @ w_g2 @ w_fc
           + np.asarray(inputs["b_g2"], np.float32) @ w_fc
           + np.asarray(inputs["b_fc"], np.float32))
    bs[0:10, 16] = c10

    xb = x.astype(bf)
    shards = []
    for c in range(NCORES):
        s = c * PER_CORE
        if c == 0:
            xs = np.concatenate(
                [np.zeros((PROLOG, 128), bf), xb[0:PER_CORE]], axis=0)
        else:
            xs = xb[s - PROLOG : s + PER_CORE]
        shards.append(np.ascontiguousarray(xs))

    common = {"w1": w1, "w2": w2, "wst": wst, "bs": bs}
    in_maps = [{"x": shards[c], **common} for c in range(NCORES)]
    return in_maps


def _gru_np(x, w_ih, b_ih, b_hh):
    gi = x @ w_ih.T + b_ih
    ir, iz, inn = gi[:, :128], gi[:, 128:256], gi[:, 256:]
    hr, hz, hn = b_hh[:128], b_hh[128:256], b_hh[256:]
    r = 1.0 / (1.0 + np.exp(-(ir + hr)))
    z = 1.0 / (1.0 + np.exp(-(iz + hz)))
    ng = np.tanh(inn + r * hn)
    return (1.0 - z) * ng


def _fix_boundary(out, inputs, side):
    """Exact (float64) recompute of the 3 boundary rows on one side."""
    M = 16  # margin
    f8 = np.float64
    if side == "left":
        xs = np.asarray(inputs["x"], np.float32)[:M].astype(f8)
    else:
        xs = np.asarray(inputs["x"], np.float32)[-M:].astype(f8)

    def cell(x, tag):
        return _gru_np(x, np.asarray(inputs[f"w_ih_{tag}"], f8),
                       np.asarray(inputs[f"b_ih_{tag}"], f8),
                       np.asarray(inputs[f"b_hh_{tag}"], f8))

    h1 = np.concatenate([cell(xs, "f1"), cell(xs, "b1")], axis=1)
    h2 = np.concatenate([cell(h1, "f2"), cell(h1, "b2")], axis=1)

    c2, c3 = 1.0 / np.sqrt(2.0), 1.0 / np.sqrt(3.0)
    dinv = np.full(M, c3, f8)
    if side == "left":
        dinv[0] = c2
    else:
        dinv[-1] = c2

    def gcn(h, w, b):
        xw = h @ np.asarray(w, f8)
        y = dinv[:, None] * xw
        s = y.copy()
        s[:-1] += y[1:]
        s[1:] += y[:-1]
        return dinv[:, None] * s + np.asarray(b, f8)

    g1 = gcn(h2, inputs["w_g1"], inputs["b_g1"])
    g2 = gcn(g1, inputs["w_g2"], inputs["b_g2"])
    o = g2 @ np.asarray(inputs["w_fc"], f8) + np.asarray(inputs["b_fc"], f8)
    # rows > margin-3 (left) / < 3 from far edge (right) are polluted by the
    # missing neighbour at the margin cut; only the 3 true boundary rows are
    # used, and those only depend on in-margin data.
    if side == "left":
        out[0:3] = o[0:3].astype(np.float32)
    else:
        out[-3:] = o[-3:].astype(np.float32)


def kernel(**inputs):
    if "prog" not in _cache:
        _cache["prog"] = _build_program()
    nc = _cache["prog"]

    in_maps = _prep_inputs(inputs)
    res = bass_utils.run_bass_kernel_spmd(nc, in_maps, core_ids=list(range(NCORES)))

    out = np.empty((N, 10), np.float32)
    for c in range(NCORES):
        shard = np.ascontiguousarray(res.results[c]["out"].T)  # [16384, 10]
        s = c * PER_CORE
        if c == 0:
            out[0 : PER_CORE - 2] = shard[2:]
        else:
            out[s - 2 : s + PER_CORE - 2] = shard
    _fix_boundary(out, inputs, "left")
    _fix_boundary(out, inputs, "right")
    return out
